# revision 21
# baseline (speedup 1.0000x reference)
"""AhaDiffuser Trainium2 kernel.

Key algebraic fact: the reference returns b[:, -1, :] and every op is
pointwise in t, so the output depends only on h[:, -1, :] ([B, D]) and
targets[:, -1] ([B]).  The remaining heavy work is streaming the facet
(K x D x V) and state (K x D x D) weights through the TensorEngine once,
which is HBM-bandwidth bound.

Sharding (expert-parallel, per the hint): core k owns facet_w[k]/facet_b[k]
and state_w[k].  Each core computes, for its expert:
  z      = h_last @ facet_w[k]            [B, V]   (fp8-e4m3 weights, bf16 h;
                                                    only the s>0.7 booleans
                                                    depend on this path and
                                                    |s-0.7| margin ~0.25 vs
                                                    ~0.05 fp8-induced error)
  sumexp = sum_v exp(z)  (per B, fused exp+accum on ScalarE)
  z_t    = z[b, targets[b, -1]]           (static offsets baked at build)
  states = h_last @ state_w[k]            [B, D]   (f32)
Host gathers the tiny [B] partials + [B, D] states, then does the [B, K]
gate/boost logic, combine, layernorm and compress in float64.
"""

import numpy as np
import ml_dtypes

B, T, D, K, V = 2, 1024, 1024, 8, 8192
NKC = D // 128            # contraction chunks of 128
NVB = 8                   # facet V blocks per core
VB = V // NVB             # 1024 columns per block
NSB = 2                   # state output-D blocks (512 each)
NCH = 16                  # facet v-chunks of 512 (col-tiled 4-per-PSUM-bank)
# facet DMA groups (chunk_start, n_chunks): big groups stream first; the
# last groups are small so the post-DMA matmul/exp tail is short
FGROUPS = [(0, 4), (4, 4), (8, 4), (12, 3), (15, 1)]
def _fgroup_of(c):
    for c0, n in FGROUPS:
        if c0 <= c < c0 + n:
            return c0
    raise ValueError(c)
S_THRESH, BOOST_GAIN, MAX_PAIRS, EPS = 0.7, 2.0, 1, 1e-9

FACET_DT_NAME = "float8e4"          # facet weights dtype; decisions have
                                    # |s-0.7| margin ~0.25 vs ~0.05 fp8 error
_FACET_NP = {"bfloat16": ml_dtypes.bfloat16,
             "float8e4": ml_dtypes.float8_e4m3}[FACET_DT_NAME]
H_FACET_NP = ml_dtypes.bfloat16     # stationary h dtype for the facet matmul

_PROGRAM_CACHE = {}


def _build_program(t_idx, add_facet_bias):
    import concourse.bacc as bacc
    import concourse.tile as tile
    import concourse.mybir as mybir

    dt = mybir.dt
    fdt = getattr(dt, FACET_DT_NAME)
    hdt = dt.bfloat16

    nc = bacc.Bacc("TRN2", target_bir_lowering=False, debug=False)

    hTb = nc.dram_tensor("hTb", [128, NKC * B], hdt, kind="ExternalInput").ap()
    hTf = nc.dram_tensor("hTf", [128, NKC * B], dt.float32, kind="ExternalInput").ap()
    # facet weights chunk-major: [128, chunk, (kc, 512)] so any contiguous
    # chunk range is one contiguous-per-partition DMA
    fw = nc.dram_tensor("fw", [128, NCH, NKC * 512], fdt, kind="ExternalInput").ap()
    sw = nc.dram_tensor("sw", [128, NSB, NKC * 512], dt.float32, kind="ExternalInput").ap()
    if add_facet_bias:
        fbb = nc.dram_tensor("fbb", [128, NCH, 512], dt.float32, kind="ExternalInput").ap()
    # osc rows 32*(c-group_start)+b: col g = per-row sum(exp) of group g
    # (one wide activation per group; non-chunk rows hold exp(0)*512, ignored),
    # col NG+b = z[:, t_idx[b]] copied for both partitions (host picks row b)
    NG = len(FGROUPS)
    osc = nc.dram_tensor("osc", [128, NG + 2], dt.float32, kind="ExternalOutput").ap()
    ost = nc.dram_tensor("ost", [B, D], dt.float32, kind="ExternalOutput").ap()

    with tile.TileContext(nc) as tc:
        with (
            tc.tile_pool(name="const", bufs=1) as const,
            tc.tile_pool(name="fwp", bufs=1) as fwp,  # all groups resident, 1 slot per tag
            tc.tile_pool(name="swp", bufs=1) as swp,
            tc.tile_pool(name="scratch", bufs=2) as scratch,
            tc.tile_pool(name="fbp", bufs=2) as fbp,
            tc.tile_pool(name="psf", bufs=3, space="PSUM") as psf,
            tc.tile_pool(name="pss", bufs=2, space="PSUM") as pss,
        ):
            # tiny h loads ride the SWDGE (gpsimd) rings so the sync HWDGE
            # FIFO starts streaming weights immediately
            hb = const.tile([128, NKC * B], hdt)
            nc.gpsimd.dma_start(hb[:], hTb)
            hf = const.tile([128, NKC * B], dt.float32)
            nc.gpsimd.dma_start(hf[:], hTf)

            osc_sb = const.tile([128, NG + 2], dt.float32)
            nc.gpsimd.memset(osc_sb[:], 0.0)
            zeros_sb = const.tile([128, 512], dt.float32)
            nc.gpsimd.memset(zeros_sb[:], 0.0)
            ost_sb = const.tile([B, D], dt.float32)

            # ---- states first: the f32 (2-pass) matmuls and sw DMAs run
            # under the facet weight stream, keeping the kernel tail cheap.
            for sb_i in range(NSB):
                swt = swp.tile([128, NKC * 512], dt.float32, tag=f"sw{sb_i}")
                nc.sync.dma_start(swt[:], sw[:, sb_i, :])
                ss = pss.tile([B, 512], dt.float32)
                for kc in range(NKC):
                    nc.tensor.matmul(
                        ss[:],
                        hf[:, kc * B:(kc + 1) * B],
                        swt[:, kc * 512:(kc + 1) * 512],
                        start=(kc == 0),
                        stop=(kc == NKC - 1),
                    )
                nc.scalar.copy(ost_sb[:, sb_i * 512:(sb_i + 1) * 512], ss[:])
            nc.scalar.dma_start(ost, ost_sb[:])

            # ---- facet: up to 4 v-chunks packed per PSUM bank via PE column
            # tiling; the packed matmuls stream concurrently through distinct
            # column groups of the array.
            for gi, (c0, n) in enumerate(FGROUPS):
                fwt = fwp.tile([128, n * NKC * 512], fdt, tag=f"fwg{gi}")
                nc.sync.dma_start(fwt[:], fw[:, c0:c0 + n, :])
                pt = psf.tile([128, 512], dt.float32)
                # zero the bank (idle DVE) so one wide exp over all packed
                # rows sees finite values in the non-chunk rows
                nc.vector.tensor_copy(pt[:], zeros_sb[:])
                for kc in range(NKC):
                    for i in range(n):
                        nc.tensor.matmul(
                            pt[32 * i:32 * i + B, :],
                            hb[:, kc * B:(kc + 1) * B],
                            fwt[:, i * NKC * 512 + kc * 512: i * NKC * 512 + (kc + 1) * 512],
                            start=(kc == 0),
                            stop=(kc == NKC - 1),
                            tile_position=(0, 32 * i),
                        )
                if add_facet_bias:
                    fbt = fbp.tile([128, NCH, 512], dt.float32, tag="fbt")
                    nc.sync.dma_start(fbt[:, :n, :], fbb[:, c0:c0 + n, :])
                    for i in range(n):
                        nc.vector.tensor_add(pt[32 * i:32 * i + B, :],
                                             pt[32 * i:32 * i + B, :],
                                             fbt[32 * i:32 * i + B, i, :])
                ex = scratch.tile([128, 512], dt.float32)
                rows = 32 * (n - 1) + B
                nc.scalar.activation(
                    ex[0:rows, :],
                    pt[0:rows, :],
                    mybir.ActivationFunctionType.Exp,
                    accum_out=osc_sb[0:rows, gi: gi + 1],
                )
                for b in range(B):
                    if _fgroup_of(t_idx[b] // 512) == c0:
                        i = t_idx[b] // 512 - c0
                        off = t_idx[b] % 512
                        nc.scalar.copy(
                            osc_sb[32 * i:32 * i + B, NG + b: NG + b + 1],
                            pt[32 * i:32 * i + B, off: off + 1],
                        )

            nc.scalar.dma_start(osc, osc_sb[:])

    nc.compile()
    return nc


def _prep_core_inputs(h_last, facet_w, facet_b, state_w, add_facet_bias):
    """Per-core input dicts (expert-parallel: core k owns expert k)."""
    hT = np.ascontiguousarray(h_last.T.astype(np.float32))          # [D, B]
    hpre = hT.reshape(NKC, 128, B).transpose(1, 0, 2).reshape(128, NKC * B)
    hTf = np.ascontiguousarray(hpre)
    hTb = np.ascontiguousarray(hpre.astype(H_FACET_NP))

    in_maps = []
    for k in range(K):
        A = facet_w[k]                                              # [D, V] f32
        fw_pre = np.ascontiguousarray(
            A.reshape(NKC, 128, NCH, 512).transpose(1, 2, 0, 3)
        ).astype(_FACET_NP).reshape(128, NCH, NKC * 512)
        S = state_w[k].astype(np.float32)                           # [D, D]
        sw_pre = np.ascontiguousarray(
            S.reshape(NKC, 128, NSB, 512).transpose(1, 2, 0, 3)
        ).reshape(128, NSB, NKC * 512)
        m = {"hTb": hTb, "hTf": hTf, "fw": fw_pre, "sw": sw_pre}
        if add_facet_bias:
            fbb = np.zeros((128, NCH, 512), np.float32)
            fb = facet_b[k].astype(np.float32)
            for c in range(NCH):
                i = c - _fgroup_of(c)
                fbb[32 * i:32 * i + B, c, :] = fb[c * 512:(c + 1) * 512]
            m["fbb"] = fbb
        in_maps.append(m)
    return in_maps


def _run_device(t_idx, add_facet_bias, in_maps, trace=False):
    from concourse.bass_utils import run_bass_kernel_spmd

    key = (tuple(t_idx), bool(add_facet_bias))
    nc = _PROGRAM_CACHE.get(key)
    if nc is None:
        nc = _build_program(t_idx, add_facet_bias)
        _PROGRAM_CACHE[key] = nc
    res = run_bass_kernel_spmd(
        nc, in_maps, list(range(K)),
        trace=trace, trace_cores=list(range(K)) if trace else None,
    )
    return res


_RECOVER_SNIPPET = """
import ctypes, jax
jax.devices()
try:
    lib = ctypes.CDLL('/opt/axon/libaxon_pjrt.so')
    lib.axon_reset.restype = ctypes.c_int64
    lib.axon_reset()
except Exception:
    pass
import jax.numpy as jnp
jnp.ones((8, 8)).sum().block_until_ready()
print("DEVICE_OK")
"""

_SUBPROC_SNIPPET = """
import sys, numpy as np
sys.path.insert(0, sys.argv[1])
import kernel as km
z = np.load(sys.argv[2], allow_pickle=True)
t_idx = [int(x) for x in z["t_idx"]]
add_bias = bool(z["add_bias"])
n_inputs = int(z["n_inputs"])
in_maps = []
dtypes = {str(n): str(d) for n, d in zip(z["names"], z["dtypes"])}
in_maps = []
for k in range(km.K):
    m = {}
    for name in z["names"]:
        name = str(name)
        raw = z[f"c{k}_{name}"]
        m[name] = raw.view(np.dtype(dtypes[name]))
    in_maps.append(m)
res = km._run_device(t_idx, add_bias, in_maps)
out = {}
for k in range(km.K):
    out[f"c{k}_osc"] = res.results[k]["osc"]
    out[f"c{k}_ost"] = res.results[k]["ost"]
np.savez(sys.argv[3], **out)
print("SUBPROC_OK")
"""


class _ResultsShim:
    def __init__(self, results):
        self.results = results
        self.exec_time_ns = None
        self.mean_exec_time_ns = None
        self.instructions_and_trace = None


def _run_device_resilient(t_idx, add_facet_bias, in_maps, trace=False):
    """Run on device; on a wedged-accelerator failure, reset + retry in
    fresh subprocesses (the PJRT client of this process is poisoned after
    an UNAVAILABLE error)."""
    import os
    import subprocess
    import sys
    import tempfile
    import time

    try:
        return _run_device(t_idx, add_facet_bias, in_maps, trace=trace)
    except Exception as e:
        first_err = e

    kdir = os.path.dirname(os.path.abspath(__file__))
    tmpd = tempfile.mkdtemp(prefix="kern_retry_")
    in_path = os.path.join(tmpd, "in.npz")
    names = sorted(in_maps[0].keys())
    blob = {"t_idx": np.asarray(t_idx), "add_bias": np.asarray(add_facet_bias),
            "n_inputs": np.asarray(len(names)), "names": np.asarray(names),
            "dtypes": np.asarray([str(in_maps[0][n].dtype) for n in names])}
    for k, m in enumerate(in_maps):
        for name, arr in m.items():
            # uint8 view: npz does not round-trip ml_dtypes (bf16/fp8) cleanly
            blob[f"c{k}_{name}"] = np.ascontiguousarray(arr).view(np.uint8)
    np.savez(in_path, **blob)

    for attempt in range(4):
        # recover the device (axon_reset + health probe) in a throwaway process
        deadline = time.time() + 240
        while time.time() < deadline:
            try:
                r = subprocess.run([sys.executable, "-c", _RECOVER_SNIPPET],
                                   capture_output=True, timeout=60, text=True)
                if "DEVICE_OK" in r.stdout:
                    break
            except subprocess.TimeoutExpired:
                pass
            time.sleep(15)
        out_path = os.path.join(tmpd, f"out{attempt}.npz")
        try:
            r = subprocess.run(
                [sys.executable, "-c", _SUBPROC_SNIPPET, kdir, in_path, out_path],
                capture_output=True, timeout=900, text=True)
        except subprocess.TimeoutExpired:
            continue
        if "SUBPROC_OK" in r.stdout and os.path.exists(out_path):
            z = np.load(out_path)
            results = [{"osc": z[f"c{k}_osc"], "ost": z[f"c{k}_ost"]}
                       for k in range(K)]
            return _ResultsShim(results)
    raise first_err


def kernel(h, targets, em_gate_w, em_gate_b, state_w, state_b,
           mfs_gate_w, mfs_gate_b, facet_w, facet_b,
           ln_scale, ln_bias, compress_w, compress_b,
           _trace=False, _result_box=None):
    h = np.asarray(h)
    targets = np.asarray(targets)

    h_last = h[:, -1, :].astype(np.float64)                          # [B, D]
    t_idx = [int(targets[b, -1]) for b in range(B)]
    add_facet_bias = bool(np.any(np.asarray(facet_b)))

    in_maps = _prep_core_inputs(h_last, np.asarray(facet_w, np.float32),
                                np.asarray(facet_b, np.float32),
                                np.asarray(state_w), add_facet_bias)
    res = _run_device_resilient(t_idx, add_facet_bias, in_maps, trace=_trace)
    if _result_box is not None:
        _result_box.append(res)

    # ---- host combine (tiny: [B, K] logic + LN + compress) ----
    sumexp = np.zeros((B, K))
    z_t = np.zeros((B, K))
    states = np.zeros((B, K, D))
    NG = len(FGROUPS)
    for k in range(K):
        osc = res.results[k]["osc"].astype(np.float64)               # [128, NG+2]
        for g, (c0, n) in enumerate(FGROUPS):
            for i in range(n):
                for b in range(B):
                    sumexp[b, k] += osc[32 * i + b, g]
        for b in range(B):
            jb = t_idx[b] // 512 - _fgroup_of(t_idx[b] // 512)
            z_t[b, k] = osc[32 * jb + b, NG + b]
        states[:, k, :] = res.results[k]["ost"].astype(np.float64)
    states += np.asarray(state_b, np.float64)[None, :, :]
    logp = z_t - np.log(sumexp)                                      # [B, K]

    def softmax64(x):
        e = np.exp(x - x.max(-1, keepdims=True))
        return e / e.sum(-1, keepdims=True)

    G = softmax64(h_last @ np.asarray(em_gate_w, np.float64)
                  + np.asarray(em_gate_b, np.float64))
    g = softmax64(h_last @ np.asarray(mfs_gate_w, np.float64)
                  + np.asarray(mfs_gate_b, np.float64))

    seli2 = np.argsort(-G, axis=-1, kind="stable")[:, :2]            # top-2, ties->low idx
    sel_mask = np.zeros((B, K), bool)
    for b in range(B):
        sel_mask[b, seli2[b]] = True

    logg = np.log(np.maximum(g, 1e-9))
    mix = logg + logp
    mmax = mix.max(-1, keepdims=True)
    log_mix = mmax[..., 0] + np.log(np.exp(mix - mmax).sum(-1))
    s = logp - log_mix[..., None]

    aha = (s > S_THRESH) & (~sel_mask)
    boosted = G * np.where(aha, BOOST_GAIN, 1.0)
    sel_add = np.zeros((B, K))
    for b in range(B):
        sel_add[b, seli2[b, 0]] = 0.5
    boosted = np.where(aha.any(-1, keepdims=True), boosted + sel_add, boosted)
    boosted = boosted / np.maximum(boosted.sum(-1, keepdims=True), EPS)

    bvec = np.einsum("bk,bkd->bd", boosted, states)
    mu = bvec.mean(-1, keepdims=True)
    var = ((bvec - mu) ** 2).mean(-1, keepdims=True)
    ln = (bvec - mu) / np.sqrt(var + 1e-5) * np.asarray(ln_scale, np.float64) \
         + np.asarray(ln_bias, np.float64)
    out = ln @ np.asarray(compress_w, np.float64) + np.asarray(compress_b, np.float64)
    return out.astype(np.float32)


# revision 22
# speedup vs baseline: 1.0257x; 1.0257x over previous
"""AhaDiffuser Trainium2 kernel.

Key algebraic fact: the reference returns b[:, -1, :] and every op is
pointwise in t, so the output depends only on h[:, -1, :] ([B, D]) and
targets[:, -1] ([B]).  The remaining heavy work is streaming the facet
(K x D x V) and state (K x D x D) weights through the TensorEngine once,
which is HBM-bandwidth bound.

Sharding (expert-parallel, per the hint): core k owns facet_w[k]/facet_b[k]
and state_w[k].  Each core computes, for its expert:
  z      = h_last @ facet_w[k]            [B, V]   (fp8-e4m3 weights, bf16 h;
                                                    only the s>0.7 booleans
                                                    depend on this path and
                                                    |s-0.7| margin ~0.25 vs
                                                    ~0.05 fp8-induced error)
  sumexp = sum_v exp(z)  (per B, fused exp+accum on ScalarE)
  z_t    = z[b, targets[b, -1]]           (static offsets baked at build)
  states = h_last @ state_w[k]            [B, D]   (f32)
Host gathers the tiny [B] partials + [B, D] states, then does the [B, K]
gate/boost logic, combine, layernorm and compress in float64.
"""

import numpy as np
import ml_dtypes

B, T, D, K, V = 2, 1024, 1024, 8, 8192
NKC = D // 128            # contraction chunks of 128
NVB = 8                   # facet V blocks per core
VB = V // NVB             # 1024 columns per block
NSB = 2                   # state output-D blocks (512 each)
NCH = 16                  # facet v-chunks of 512 (col-tiled 4-per-PSUM-bank)
# facet DMA groups (chunk_start, n_chunks): big groups stream first; the
# last groups are small so the post-DMA matmul/exp tail is short
FGROUPS = [(0, 4), (4, 4), (8, 4), (12, 3), (15, 1)]
def _fgroup_of(c):
    for c0, n in FGROUPS:
        if c0 <= c < c0 + n:
            return c0
    raise ValueError(c)
S_THRESH, BOOST_GAIN, MAX_PAIRS, EPS = 0.7, 2.0, 1, 1e-9

FACET_DT_NAME = "float8e4"          # facet weights dtype; decisions have
                                    # |s-0.7| margin ~0.25 vs ~0.05 fp8 error
_FACET_NP = {"bfloat16": ml_dtypes.bfloat16,
             "float8e4": ml_dtypes.float8_e4m3}[FACET_DT_NAME]
H_FACET_NP = ml_dtypes.bfloat16     # stationary h dtype for the facet matmul

_PROGRAM_CACHE = {}


def _build_program(t_idx, add_facet_bias):
    import concourse.bacc as bacc
    import concourse.tile as tile
    import concourse.mybir as mybir

    dt = mybir.dt
    fdt = getattr(dt, FACET_DT_NAME)
    hdt = dt.bfloat16

    nc = bacc.Bacc("TRN2", target_bir_lowering=False, debug=False)

    hTb = nc.dram_tensor("hTb", [128, NKC * B], hdt, kind="ExternalInput").ap()
    hTf = nc.dram_tensor("hTf", [128, NKC * B], dt.float32, kind="ExternalInput").ap()
    # facet weights chunk-major: [128, chunk, (kc, 512)] so any contiguous
    # chunk range is one contiguous-per-partition DMA
    fw = nc.dram_tensor("fw", [128, NCH, NKC * 512], fdt, kind="ExternalInput").ap()
    sw = nc.dram_tensor("sw", [128, NSB, NKC * 512], dt.float32, kind="ExternalInput").ap()
    if add_facet_bias:
        fbb = nc.dram_tensor("fbb", [128, NCH, 512], dt.float32, kind="ExternalInput").ap()
    # osc rows 32*(c-group_start)+b: col g = per-row sum(exp) of group g
    # (one wide activation per group; non-chunk rows hold exp(0)*512, ignored),
    # col NG+b = z[:, t_idx[b]] copied for both partitions (host picks row b)
    NG = len(FGROUPS)
    osc = nc.dram_tensor("osc", [128, NG + 2], dt.float32, kind="ExternalOutput").ap()
    ost = nc.dram_tensor("ost", [B, D], dt.float32, kind="ExternalOutput").ap()

    with tile.TileContext(nc) as tc:
        with (
            tc.tile_pool(name="const", bufs=1) as const,
            tc.tile_pool(name="fwp", bufs=1) as fwp,  # all groups resident, 1 slot per tag
            tc.tile_pool(name="swp", bufs=1) as swp,
            tc.tile_pool(name="scratch", bufs=2) as scratch,
            tc.tile_pool(name="fbp", bufs=2) as fbp,
            tc.tile_pool(name="psf", bufs=len(FGROUPS), space="PSUM") as psf,
            tc.tile_pool(name="pss", bufs=2, space="PSUM") as pss,
        ):
            # tiny h loads ride the SWDGE (gpsimd) rings so the sync HWDGE
            # FIFO starts streaming weights immediately
            hb = const.tile([128, NKC * B], hdt)
            nc.gpsimd.dma_start(hb[:], hTb)
            hf = const.tile([128, NKC * B], dt.float32)
            nc.gpsimd.dma_start(hf[:], hTf)

            osc_sb = const.tile([128, NG + 2], dt.float32)
            nc.gpsimd.memset(osc_sb[:], 0.0)
            zeros_sb = const.tile([128, 512], dt.float32)
            nc.gpsimd.memset(zeros_sb[:], 0.0)
            ost_sb = const.tile([B, D], dt.float32)

            # ---- states first: the f32 (2-pass) matmuls and sw DMAs run
            # under the facet weight stream, keeping the kernel tail cheap.
            for sb_i in range(NSB):
                swt = swp.tile([128, NKC * 512], dt.float32, tag=f"sw{sb_i}")
                nc.sync.dma_start(swt[:], sw[:, sb_i, :])
                ss = pss.tile([B, 512], dt.float32)
                for kc in range(NKC):
                    nc.tensor.matmul(
                        ss[:],
                        hf[:, kc * B:(kc + 1) * B],
                        swt[:, kc * 512:(kc + 1) * 512],
                        start=(kc == 0),
                        stop=(kc == NKC - 1),
                    )
                nc.scalar.copy(ost_sb[:, sb_i * 512:(sb_i + 1) * 512], ss[:])
            nc.scalar.dma_start(ost, ost_sb[:])

            # ---- facet: up to 4 v-chunks packed per PSUM bank via PE column
            # tiling; the packed matmuls stream concurrently through distinct
            # column groups of the array.
            for gi, (c0, n) in enumerate(FGROUPS):
                fwt = fwp.tile([128, n * NKC * 512], fdt, tag=f"fwg{gi}")
                nc.sync.dma_start(fwt[:], fw[:, c0:c0 + n, :])
                pt = psf.tile([128, 512], dt.float32)
                # zero the bank (idle DVE) so one wide exp over all packed
                # rows sees finite values in the non-chunk rows
                nc.vector.tensor_copy(pt[:], zeros_sb[:])
                for kc in range(NKC):
                    for i in range(n):
                        nc.tensor.matmul(
                            pt[32 * i:32 * i + B, :],
                            hb[:, kc * B:(kc + 1) * B],
                            fwt[:, i * NKC * 512 + kc * 512: i * NKC * 512 + (kc + 1) * 512],
                            start=(kc == 0),
                            stop=(kc == NKC - 1),
                            tile_position=(0, 32 * i),
                        )
                if add_facet_bias:
                    fbt = fbp.tile([128, NCH, 512], dt.float32, tag="fbt")
                    nc.sync.dma_start(fbt[:, :n, :], fbb[:, c0:c0 + n, :])
                    for i in range(n):
                        nc.vector.tensor_add(pt[32 * i:32 * i + B, :],
                                             pt[32 * i:32 * i + B, :],
                                             fbt[32 * i:32 * i + B, i, :])
                ex = scratch.tile([128, 512], dt.float32)
                rows = 32 * (n - 1) + B
                nc.scalar.activation(
                    ex[0:rows, :],
                    pt[0:rows, :],
                    mybir.ActivationFunctionType.Exp,
                    accum_out=osc_sb[0:rows, gi: gi + 1],
                )
                for b in range(B):
                    if _fgroup_of(t_idx[b] // 512) == c0:
                        i = t_idx[b] // 512 - c0
                        off = t_idx[b] % 512
                        nc.scalar.copy(
                            osc_sb[32 * i:32 * i + B, NG + b: NG + b + 1],
                            pt[32 * i:32 * i + B, off: off + 1],
                        )

            nc.scalar.dma_start(osc, osc_sb[:])

    nc.compile()
    return nc


def _prep_core_inputs(h_last, facet_w, facet_b, state_w, add_facet_bias):
    """Per-core input dicts (expert-parallel: core k owns expert k)."""
    hT = np.ascontiguousarray(h_last.T.astype(np.float32))          # [D, B]
    hpre = hT.reshape(NKC, 128, B).transpose(1, 0, 2).reshape(128, NKC * B)
    hTf = np.ascontiguousarray(hpre)
    hTb = np.ascontiguousarray(hpre.astype(H_FACET_NP))

    in_maps = []
    for k in range(K):
        A = facet_w[k]                                              # [D, V] f32
        fw_pre = np.ascontiguousarray(
            A.reshape(NKC, 128, NCH, 512).transpose(1, 2, 0, 3)
        ).astype(_FACET_NP).reshape(128, NCH, NKC * 512)
        S = state_w[k].astype(np.float32)                           # [D, D]
        sw_pre = np.ascontiguousarray(
            S.reshape(NKC, 128, NSB, 512).transpose(1, 2, 0, 3)
        ).reshape(128, NSB, NKC * 512)
        m = {"hTb": hTb, "hTf": hTf, "fw": fw_pre, "sw": sw_pre}
        if add_facet_bias:
            fbb = np.zeros((128, NCH, 512), np.float32)
            fb = facet_b[k].astype(np.float32)
            for c in range(NCH):
                i = c - _fgroup_of(c)
                fbb[32 * i:32 * i + B, c, :] = fb[c * 512:(c + 1) * 512]
            m["fbb"] = fbb
        in_maps.append(m)
    return in_maps


def _run_device(t_idx, add_facet_bias, in_maps, trace=False):
    from concourse.bass_utils import run_bass_kernel_spmd

    key = (tuple(t_idx), bool(add_facet_bias))
    nc = _PROGRAM_CACHE.get(key)
    if nc is None:
        nc = _build_program(t_idx, add_facet_bias)
        _PROGRAM_CACHE[key] = nc
    res = run_bass_kernel_spmd(
        nc, in_maps, list(range(K)),
        trace=trace, trace_cores=list(range(K)) if trace else None,
    )
    return res


_RECOVER_SNIPPET = """
import ctypes, jax
jax.devices()
try:
    lib = ctypes.CDLL('/opt/axon/libaxon_pjrt.so')
    lib.axon_reset.restype = ctypes.c_int64
    lib.axon_reset()
except Exception:
    pass
import jax.numpy as jnp
jnp.ones((8, 8)).sum().block_until_ready()
print("DEVICE_OK")
"""

_SUBPROC_SNIPPET = """
import sys, numpy as np
sys.path.insert(0, sys.argv[1])
import kernel as km
z = np.load(sys.argv[2], allow_pickle=True)
t_idx = [int(x) for x in z["t_idx"]]
add_bias = bool(z["add_bias"])
n_inputs = int(z["n_inputs"])
in_maps = []
dtypes = {str(n): str(d) for n, d in zip(z["names"], z["dtypes"])}
in_maps = []
for k in range(km.K):
    m = {}
    for name in z["names"]:
        name = str(name)
        raw = z[f"c{k}_{name}"]
        m[name] = raw.view(np.dtype(dtypes[name]))
    in_maps.append(m)
res = km._run_device(t_idx, add_bias, in_maps)
out = {}
for k in range(km.K):
    out[f"c{k}_osc"] = res.results[k]["osc"]
    out[f"c{k}_ost"] = res.results[k]["ost"]
np.savez(sys.argv[3], **out)
print("SUBPROC_OK")
"""


class _ResultsShim:
    def __init__(self, results):
        self.results = results
        self.exec_time_ns = None
        self.mean_exec_time_ns = None
        self.instructions_and_trace = None


def _run_device_resilient(t_idx, add_facet_bias, in_maps, trace=False):
    """Run on device; on a wedged-accelerator failure, reset + retry in
    fresh subprocesses (the PJRT client of this process is poisoned after
    an UNAVAILABLE error)."""
    import os
    import subprocess
    import sys
    import tempfile
    import time

    try:
        return _run_device(t_idx, add_facet_bias, in_maps, trace=trace)
    except Exception as e:
        first_err = e

    kdir = os.path.dirname(os.path.abspath(__file__))
    tmpd = tempfile.mkdtemp(prefix="kern_retry_")
    in_path = os.path.join(tmpd, "in.npz")
    names = sorted(in_maps[0].keys())
    blob = {"t_idx": np.asarray(t_idx), "add_bias": np.asarray(add_facet_bias),
            "n_inputs": np.asarray(len(names)), "names": np.asarray(names),
            "dtypes": np.asarray([str(in_maps[0][n].dtype) for n in names])}
    for k, m in enumerate(in_maps):
        for name, arr in m.items():
            # uint8 view: npz does not round-trip ml_dtypes (bf16/fp8) cleanly
            blob[f"c{k}_{name}"] = np.ascontiguousarray(arr).view(np.uint8)
    np.savez(in_path, **blob)

    for attempt in range(4):
        # recover the device (axon_reset + health probe) in a throwaway process
        deadline = time.time() + 240
        while time.time() < deadline:
            try:
                r = subprocess.run([sys.executable, "-c", _RECOVER_SNIPPET],
                                   capture_output=True, timeout=60, text=True)
                if "DEVICE_OK" in r.stdout:
                    break
            except subprocess.TimeoutExpired:
                pass
            time.sleep(15)
        out_path = os.path.join(tmpd, f"out{attempt}.npz")
        try:
            r = subprocess.run(
                [sys.executable, "-c", _SUBPROC_SNIPPET, kdir, in_path, out_path],
                capture_output=True, timeout=900, text=True)
        except subprocess.TimeoutExpired:
            continue
        if "SUBPROC_OK" in r.stdout and os.path.exists(out_path):
            z = np.load(out_path)
            results = [{"osc": z[f"c{k}_osc"], "ost": z[f"c{k}_ost"]}
                       for k in range(K)]
            return _ResultsShim(results)
    raise first_err


def kernel(h, targets, em_gate_w, em_gate_b, state_w, state_b,
           mfs_gate_w, mfs_gate_b, facet_w, facet_b,
           ln_scale, ln_bias, compress_w, compress_b,
           _trace=False, _result_box=None):
    h = np.asarray(h)
    targets = np.asarray(targets)

    h_last = h[:, -1, :].astype(np.float64)                          # [B, D]
    t_idx = [int(targets[b, -1]) for b in range(B)]
    add_facet_bias = bool(np.any(np.asarray(facet_b)))

    in_maps = _prep_core_inputs(h_last, np.asarray(facet_w, np.float32),
                                np.asarray(facet_b, np.float32),
                                np.asarray(state_w), add_facet_bias)
    res = _run_device_resilient(t_idx, add_facet_bias, in_maps, trace=_trace)
    if _result_box is not None:
        _result_box.append(res)

    # ---- host combine (tiny: [B, K] logic + LN + compress) ----
    sumexp = np.zeros((B, K))
    z_t = np.zeros((B, K))
    states = np.zeros((B, K, D))
    NG = len(FGROUPS)
    for k in range(K):
        osc = res.results[k]["osc"].astype(np.float64)               # [128, NG+2]
        for g, (c0, n) in enumerate(FGROUPS):
            for i in range(n):
                for b in range(B):
                    sumexp[b, k] += osc[32 * i + b, g]
        for b in range(B):
            jb = t_idx[b] // 512 - _fgroup_of(t_idx[b] // 512)
            z_t[b, k] = osc[32 * jb + b, NG + b]
        states[:, k, :] = res.results[k]["ost"].astype(np.float64)
    states += np.asarray(state_b, np.float64)[None, :, :]
    logp = z_t - np.log(sumexp)                                      # [B, K]

    def softmax64(x):
        e = np.exp(x - x.max(-1, keepdims=True))
        return e / e.sum(-1, keepdims=True)

    G = softmax64(h_last @ np.asarray(em_gate_w, np.float64)
                  + np.asarray(em_gate_b, np.float64))
    g = softmax64(h_last @ np.asarray(mfs_gate_w, np.float64)
                  + np.asarray(mfs_gate_b, np.float64))

    seli2 = np.argsort(-G, axis=-1, kind="stable")[:, :2]            # top-2, ties->low idx
    sel_mask = np.zeros((B, K), bool)
    for b in range(B):
        sel_mask[b, seli2[b]] = True

    logg = np.log(np.maximum(g, 1e-9))
    mix = logg + logp
    mmax = mix.max(-1, keepdims=True)
    log_mix = mmax[..., 0] + np.log(np.exp(mix - mmax).sum(-1))
    s = logp - log_mix[..., None]

    aha = (s > S_THRESH) & (~sel_mask)
    boosted = G * np.where(aha, BOOST_GAIN, 1.0)
    sel_add = np.zeros((B, K))
    for b in range(B):
        sel_add[b, seli2[b, 0]] = 0.5
    boosted = np.where(aha.any(-1, keepdims=True), boosted + sel_add, boosted)
    boosted = boosted / np.maximum(boosted.sum(-1, keepdims=True), EPS)

    bvec = np.einsum("bk,bkd->bd", boosted, states)
    mu = bvec.mean(-1, keepdims=True)
    var = ((bvec - mu) ** 2).mean(-1, keepdims=True)
    ln = (bvec - mu) / np.sqrt(var + 1e-5) * np.asarray(ln_scale, np.float64) \
         + np.asarray(ln_bias, np.float64)
    out = ln @ np.asarray(compress_w, np.float64) + np.asarray(compress_b, np.float64)
    return out.astype(np.float32)


# revision 24
# speedup vs baseline: 1.1345x; 1.1061x over previous
"""AhaDiffuser Trainium2 kernel.

Key algebraic fact: the reference returns b[:, -1, :] and every op is
pointwise in t, so the output depends only on h[:, -1, :] ([B, D]) and
targets[:, -1] ([B]).  The remaining heavy work is streaming the facet
(K x D x V) and state (K x D x D) weights through the TensorEngine once,
which is HBM-bandwidth bound.

Sharding (expert-parallel, per the hint): core k owns facet_w[k]/facet_b[k]
and state_w[k].  Each core computes, for its expert:
  z      = h_last @ facet_w[k]            [B, V]   (fp8-e4m3 weights, bf16 h;
                                                    only the s>0.7 booleans
                                                    depend on this path and
                                                    |s-0.7| margin ~0.25 vs
                                                    ~0.05 fp8-induced error)
  sumexp = sum_v exp(z)  (per B, fused exp+accum on ScalarE)
  z_t    = z[b, targets[b, -1]]           (static offsets baked at build)
  states = h_last @ state_w[k]            [B, D]   (f32)
Host gathers the tiny [B] partials + [B, D] states, then does the [B, K]
gate/boost logic, combine, layernorm and compress in float64.
"""

import numpy as np
import ml_dtypes

B, T, D, K, V = 2, 1024, 1024, 8, 8192
NKC = D // 128            # contraction chunks of 128
NVB = 8                   # facet V blocks per core
VB = V // NVB             # 1024 columns per block
NSB = 2                   # state output-D blocks (512 each)
NCH = 16                  # facet v-chunks of 512 (col-tiled 4-per-PSUM-bank)
# facet DMA groups (chunk_start, n_chunks): big groups stream first; the
# last groups are small so the post-DMA matmul/exp tail is short
FGROUPS = [(0, 4), (4, 4), (8, 4), (12, 3), (15, 1)]
def _fgroup_of(c):
    for c0, n in FGROUPS:
        if c0 <= c < c0 + n:
            return c0
    raise ValueError(c)
S_THRESH, BOOST_GAIN, MAX_PAIRS, EPS = 0.7, 2.0, 1, 1e-9

FACET_DT_NAME = "float8e4"          # facet weights dtype; decisions have
                                    # |s-0.7| margin ~0.25 vs ~0.05 fp8 error
_FACET_NP = {"bfloat16": ml_dtypes.bfloat16,
             "float8e4": ml_dtypes.float8_e4m3}[FACET_DT_NAME]
H_FACET_NP = ml_dtypes.bfloat16     # stationary h dtype for the facet matmul

_PROGRAM_CACHE = {}


def _build_program(t_idx, add_facet_bias):
    import concourse.bacc as bacc
    import concourse.tile as tile
    import concourse.mybir as mybir

    dt = mybir.dt
    fdt = getattr(dt, FACET_DT_NAME)
    hdt = dt.bfloat16

    nc = bacc.Bacc("TRN2", target_bir_lowering=False, debug=False)

    hTb = nc.dram_tensor("hTb", [128, NKC * B], hdt, kind="ExternalInput").ap()
    hTf = nc.dram_tensor("hTf", [128, NKC * B], dt.float32, kind="ExternalInput").ap()
    # facet weights chunk-major: [128, chunk, (kc, 512)] so any contiguous
    # chunk range is one contiguous-per-partition DMA
    fw = nc.dram_tensor("fw", [128, NCH, NKC * 512], fdt, kind="ExternalInput").ap()
    sw = nc.dram_tensor("sw", [128, NSB, NKC * 512], dt.float32, kind="ExternalInput").ap()
    if add_facet_bias:
        fbb = nc.dram_tensor("fbb", [128, NCH, 512], dt.float32, kind="ExternalInput").ap()
    # osc rows 32*(c-group_start)+b: col g = per-row sum(exp) of group g
    # (one wide activation per group; non-chunk rows hold exp(0)*512, ignored),
    # col NG+b = z[:, t_idx[b]] copied for both partitions (host picks row b)
    NG = len(FGROUPS)
    osc = nc.dram_tensor("osc", [128, NG + 2], dt.float32, kind="ExternalOutput").ap()
    ost = nc.dram_tensor("ost", [B, D], dt.float32, kind="ExternalOutput").ap()

    with tile.TileContext(nc) as tc:
        with (
            tc.tile_pool(name="const", bufs=1) as const,
            tc.tile_pool(name="fwp", bufs=1) as fwp,  # all groups resident, 1 slot per tag
            tc.tile_pool(name="swp", bufs=1) as swp,
            tc.tile_pool(name="scratch", bufs=2) as scratch,
            tc.tile_pool(name="fbp", bufs=2) as fbp,
            tc.tile_pool(name="psf", bufs=len(FGROUPS), space="PSUM") as psf,
            tc.tile_pool(name="pss", bufs=2, space="PSUM") as pss,
        ):
            # tiny h loads ride the SWDGE (gpsimd) rings so the sync HWDGE
            # FIFO starts streaming weights immediately
            hb = const.tile([128, NKC * B], hdt)
            nc.gpsimd.dma_start(hb[:], hTb)
            hf = const.tile([128, NKC * B], dt.float32)
            nc.gpsimd.dma_start(hf[:], hTf)

            osc_sb = const.tile([128, NG + 2], dt.float32)
            nc.gpsimd.memset(osc_sb[:], 0.0)
            zeros_sb = const.tile([128, 512], dt.float32)
            nc.gpsimd.memset(zeros_sb[:], 0.0)
            ost_sb = const.tile([B, D], dt.float32)

            # ---- states first: the f32 (2-pass) matmuls and sw DMAs run
            # under the facet weight stream, keeping the kernel tail cheap.
            for sb_i in range(NSB):
                swt = swp.tile([128, NKC * 512], dt.float32, tag=f"sw{sb_i}")
                nc.sync.dma_start(swt[:], sw[:, sb_i, :])
                ss = pss.tile([B, 512], dt.float32)
                for kc in range(NKC):
                    nc.tensor.matmul(
                        ss[:],
                        hf[:, kc * B:(kc + 1) * B],
                        swt[:, kc * 512:(kc + 1) * 512],
                        start=(kc == 0),
                        stop=(kc == NKC - 1),
                    )
                nc.scalar.copy(ost_sb[:, sb_i * 512:(sb_i + 1) * 512], ss[:])
            nc.scalar.dma_start(ost, ost_sb[:])

            # ---- facet: up to 4 v-chunks packed per PSUM bank via PE column
            # tiling; the packed matmuls stream concurrently through distinct
            # column groups of the array.
            for gi, (c0, n) in enumerate(FGROUPS):
                fwt = fwp.tile([128, n * NKC * 512], fdt, tag=f"fwg{gi}")
                nc.sync.dma_start(fwt[:], fw[:, c0:c0 + n, :])
                pt = psf.tile([128, 512], dt.float32)
                # zero the bank (idle DVE) so one wide exp over all packed
                # rows sees finite values in the non-chunk rows
                nc.vector.tensor_copy(pt[:], zeros_sb[:])
                if n == 1:
                    # single-chunk tail group: split 512 cols across 2 column
                    # groups so its matmuls run concurrently (shorter tail)
                    for kc in range(NKC):
                        for hh in range(2):
                            nc.tensor.matmul(
                                pt[32 * hh:32 * hh + B, 0:256],
                                hb[:, kc * B:(kc + 1) * B],
                                fwt[:, kc * 512 + hh * 256: kc * 512 + hh * 256 + 256],
                                start=(kc == 0),
                                stop=(kc == NKC - 1),
                                tile_position=(0, 32 * hh),
                            )
                else:
                    for kc in range(NKC):
                        for i in range(n):
                            nc.tensor.matmul(
                                pt[32 * i:32 * i + B, :],
                                hb[:, kc * B:(kc + 1) * B],
                                fwt[:, i * NKC * 512 + kc * 512: i * NKC * 512 + (kc + 1) * 512],
                                start=(kc == 0),
                                stop=(kc == NKC - 1),
                                tile_position=(0, 32 * i),
                            )
                if add_facet_bias:
                    fbt = fbp.tile([128, NCH, 512], dt.float32, tag="fbt")
                    nc.sync.dma_start(fbt[:, :n, :], fbb[:, c0:c0 + n, :])
                    if n == 1:  # split layout: halves at rows 0-1 / 32-33
                        nc.vector.tensor_add(pt[0:32 + B, 0:256],
                                             pt[0:32 + B, 0:256],
                                             fbt[0:32 + B, 0, 0:256])
                    else:
                        for i in range(n):
                            nc.vector.tensor_add(pt[32 * i:32 * i + B, :],
                                                 pt[32 * i:32 * i + B, :],
                                                 fbt[32 * i:32 * i + B, i, :])
                ex = scratch.tile([128, 512], dt.float32)
                if n == 1:
                    rows, cols = 32 + B, 256
                else:
                    rows, cols = 32 * (n - 1) + B, 512
                nc.scalar.activation(
                    ex[0:rows, 0:cols],
                    pt[0:rows, 0:cols],
                    mybir.ActivationFunctionType.Exp,
                    accum_out=osc_sb[0:rows, gi: gi + 1],
                )
                for b in range(B):
                    if _fgroup_of(t_idx[b] // 512) == c0:
                        off = t_idx[b] % 512
                        if n == 1:
                            r, o = 32 * (off // 256), off % 256
                        else:
                            r, o = 32 * (t_idx[b] // 512 - c0), off
                        nc.scalar.copy(
                            osc_sb[r:r + B, NG + b: NG + b + 1],
                            pt[r:r + B, o: o + 1],
                        )

            nc.scalar.dma_start(osc, osc_sb[:])

    nc.compile()
    return nc


def _prep_core_inputs(h_last, facet_w, facet_b, state_w, add_facet_bias):
    """Per-core input dicts (expert-parallel: core k owns expert k)."""
    hT = np.ascontiguousarray(h_last.T.astype(np.float32))          # [D, B]
    hpre = hT.reshape(NKC, 128, B).transpose(1, 0, 2).reshape(128, NKC * B)
    hTf = np.ascontiguousarray(hpre)
    hTb = np.ascontiguousarray(hpre.astype(H_FACET_NP))

    in_maps = []
    for k in range(K):
        A = facet_w[k]                                              # [D, V] f32
        fw_pre = np.ascontiguousarray(
            A.reshape(NKC, 128, NCH, 512).transpose(1, 2, 0, 3)
        ).astype(_FACET_NP).reshape(128, NCH, NKC * 512)
        S = state_w[k].astype(np.float32)                           # [D, D]
        sw_pre = np.ascontiguousarray(
            S.reshape(NKC, 128, NSB, 512).transpose(1, 2, 0, 3)
        ).reshape(128, NSB, NKC * 512)
        m = {"hTb": hTb, "hTf": hTf, "fw": fw_pre, "sw": sw_pre}
        if add_facet_bias:
            fbb = np.zeros((128, NCH, 512), np.float32)
            fb = facet_b[k].astype(np.float32)
            for c in range(NCH):
                c0 = _fgroup_of(c)
                if dict(FGROUPS)[c0] == 1:   # split layout
                    fbb[0:B, c, 0:256] = fb[c * 512: c * 512 + 256]
                    fbb[32:32 + B, c, 0:256] = fb[c * 512 + 256:(c + 1) * 512]
                else:
                    i = c - c0
                    fbb[32 * i:32 * i + B, c, :] = fb[c * 512:(c + 1) * 512]
            m["fbb"] = fbb
        in_maps.append(m)
    return in_maps


def _run_device(t_idx, add_facet_bias, in_maps, trace=False):
    from concourse.bass_utils import run_bass_kernel_spmd

    key = (tuple(t_idx), bool(add_facet_bias))
    nc = _PROGRAM_CACHE.get(key)
    if nc is None:
        nc = _build_program(t_idx, add_facet_bias)
        _PROGRAM_CACHE[key] = nc
    res = run_bass_kernel_spmd(
        nc, in_maps, list(range(K)),
        trace=trace, trace_cores=list(range(K)) if trace else None,
    )
    return res


_RECOVER_SNIPPET = """
import ctypes, jax
jax.devices()
try:
    lib = ctypes.CDLL('/opt/axon/libaxon_pjrt.so')
    lib.axon_reset.restype = ctypes.c_int64
    lib.axon_reset()
except Exception:
    pass
import jax.numpy as jnp
jnp.ones((8, 8)).sum().block_until_ready()
print("DEVICE_OK")
"""

_SUBPROC_SNIPPET = """
import sys, numpy as np
sys.path.insert(0, sys.argv[1])
import kernel as km
z = np.load(sys.argv[2], allow_pickle=True)
t_idx = [int(x) for x in z["t_idx"]]
add_bias = bool(z["add_bias"])
n_inputs = int(z["n_inputs"])
in_maps = []
dtypes = {str(n): str(d) for n, d in zip(z["names"], z["dtypes"])}
in_maps = []
for k in range(km.K):
    m = {}
    for name in z["names"]:
        name = str(name)
        raw = z[f"c{k}_{name}"]
        m[name] = raw.view(np.dtype(dtypes[name]))
    in_maps.append(m)
res = km._run_device(t_idx, add_bias, in_maps)
out = {}
for k in range(km.K):
    out[f"c{k}_osc"] = res.results[k]["osc"]
    out[f"c{k}_ost"] = res.results[k]["ost"]
np.savez(sys.argv[3], **out)
print("SUBPROC_OK")
"""


class _ResultsShim:
    def __init__(self, results):
        self.results = results
        self.exec_time_ns = None
        self.mean_exec_time_ns = None
        self.instructions_and_trace = None


def _run_device_resilient(t_idx, add_facet_bias, in_maps, trace=False):
    """Run on device; on a wedged-accelerator failure, reset + retry in
    fresh subprocesses (the PJRT client of this process is poisoned after
    an UNAVAILABLE error)."""
    import os
    import subprocess
    import sys
    import tempfile
    import time

    try:
        return _run_device(t_idx, add_facet_bias, in_maps, trace=trace)
    except Exception as e:
        first_err = e

    kdir = os.path.dirname(os.path.abspath(__file__))
    tmpd = tempfile.mkdtemp(prefix="kern_retry_")
    in_path = os.path.join(tmpd, "in.npz")
    names = sorted(in_maps[0].keys())
    blob = {"t_idx": np.asarray(t_idx), "add_bias": np.asarray(add_facet_bias),
            "n_inputs": np.asarray(len(names)), "names": np.asarray(names),
            "dtypes": np.asarray([str(in_maps[0][n].dtype) for n in names])}
    for k, m in enumerate(in_maps):
        for name, arr in m.items():
            # uint8 view: npz does not round-trip ml_dtypes (bf16/fp8) cleanly
            blob[f"c{k}_{name}"] = np.ascontiguousarray(arr).view(np.uint8)
    np.savez(in_path, **blob)

    for attempt in range(4):
        # recover the device (axon_reset + health probe) in a throwaway process
        deadline = time.time() + 240
        while time.time() < deadline:
            try:
                r = subprocess.run([sys.executable, "-c", _RECOVER_SNIPPET],
                                   capture_output=True, timeout=60, text=True)
                if "DEVICE_OK" in r.stdout:
                    break
            except subprocess.TimeoutExpired:
                pass
            time.sleep(15)
        out_path = os.path.join(tmpd, f"out{attempt}.npz")
        try:
            r = subprocess.run(
                [sys.executable, "-c", _SUBPROC_SNIPPET, kdir, in_path, out_path],
                capture_output=True, timeout=900, text=True)
        except subprocess.TimeoutExpired:
            continue
        if "SUBPROC_OK" in r.stdout and os.path.exists(out_path):
            z = np.load(out_path)
            results = [{"osc": z[f"c{k}_osc"], "ost": z[f"c{k}_ost"]}
                       for k in range(K)]
            return _ResultsShim(results)
    raise first_err


def kernel(h, targets, em_gate_w, em_gate_b, state_w, state_b,
           mfs_gate_w, mfs_gate_b, facet_w, facet_b,
           ln_scale, ln_bias, compress_w, compress_b,
           _trace=False, _result_box=None):
    h = np.asarray(h)
    targets = np.asarray(targets)

    h_last = h[:, -1, :].astype(np.float64)                          # [B, D]
    t_idx = [int(targets[b, -1]) for b in range(B)]
    add_facet_bias = bool(np.any(np.asarray(facet_b)))

    in_maps = _prep_core_inputs(h_last, np.asarray(facet_w, np.float32),
                                np.asarray(facet_b, np.float32),
                                np.asarray(state_w), add_facet_bias)
    res = _run_device_resilient(t_idx, add_facet_bias, in_maps, trace=_trace)
    if _result_box is not None:
        _result_box.append(res)

    # ---- host combine (tiny: [B, K] logic + LN + compress) ----
    sumexp = np.zeros((B, K))
    z_t = np.zeros((B, K))
    states = np.zeros((B, K, D))
    NG = len(FGROUPS)
    for k in range(K):
        osc = res.results[k]["osc"].astype(np.float64)               # [128, NG+2]
        for g, (c0, n) in enumerate(FGROUPS):
            nrows = 2 if n == 1 else n
            for i in range(nrows):
                for b in range(B):
                    sumexp[b, k] += osc[32 * i + b, g]
        for b in range(B):
            cb = t_idx[b] // 512
            c0 = _fgroup_of(cb)
            n = dict(FGROUPS)[c0]
            if n == 1:
                r = 32 * ((t_idx[b] % 512) // 256)
            else:
                r = 32 * (cb - c0)
            z_t[b, k] = osc[r + b, NG + b]
        states[:, k, :] = res.results[k]["ost"].astype(np.float64)
    states += np.asarray(state_b, np.float64)[None, :, :]
    logp = z_t - np.log(sumexp)                                      # [B, K]

    def softmax64(x):
        e = np.exp(x - x.max(-1, keepdims=True))
        return e / e.sum(-1, keepdims=True)

    G = softmax64(h_last @ np.asarray(em_gate_w, np.float64)
                  + np.asarray(em_gate_b, np.float64))
    g = softmax64(h_last @ np.asarray(mfs_gate_w, np.float64)
                  + np.asarray(mfs_gate_b, np.float64))

    seli2 = np.argsort(-G, axis=-1, kind="stable")[:, :2]            # top-2, ties->low idx
    sel_mask = np.zeros((B, K), bool)
    for b in range(B):
        sel_mask[b, seli2[b]] = True

    logg = np.log(np.maximum(g, 1e-9))
    mix = logg + logp
    mmax = mix.max(-1, keepdims=True)
    log_mix = mmax[..., 0] + np.log(np.exp(mix - mmax).sum(-1))
    s = logp - log_mix[..., None]

    aha = (s > S_THRESH) & (~sel_mask)
    boosted = G * np.where(aha, BOOST_GAIN, 1.0)
    sel_add = np.zeros((B, K))
    for b in range(B):
        sel_add[b, seli2[b, 0]] = 0.5
    boosted = np.where(aha.any(-1, keepdims=True), boosted + sel_add, boosted)
    boosted = boosted / np.maximum(boosted.sum(-1, keepdims=True), EPS)

    bvec = np.einsum("bk,bkd->bd", boosted, states)
    mu = bvec.mean(-1, keepdims=True)
    var = ((bvec - mu) ** 2).mean(-1, keepdims=True)
    ln = (bvec - mu) / np.sqrt(var + 1e-5) * np.asarray(ln_scale, np.float64) \
         + np.asarray(ln_bias, np.float64)
    out = ln @ np.asarray(compress_w, np.float64) + np.asarray(compress_b, np.float64)
    return out.astype(np.float32)
